# revision 11
# baseline (speedup 1.0000x reference)
"""GCNConv (COO SpMM + feature transform) distributed over 8 NeuronCores.

out = segment_sum(x[cols] * vals, rows) @ weight

Strategy (1D row partition of the sparse matrix, per the CAGNET-style hint):
 - Destination rows are split into 8 contiguous blocks of 12500 rows; core k
   owns rows [12500k, 12500(k+1)) and the edges targeting them (edges arrive
   sorted by destination row).
 - x (the gather table) and the 32x32 weight are replicated per core.
 - Host-side (inside kernel(), numpy): each core's rows are bin-packed into
   "tiles" of <=128 edge slots / <=M_FIX rows.  For each tile we build
     idx[p]  : source node of edge-slot p   (gather index)
     bval[p, i] = val(edge) if slot p belongs to tile-row i else 0
   i.e. bval is the one-hot segment-sum matrix with the edge weights folded
   in, fully precomputed on host.
 - Device: per super-block of TPS tiles
     gather  : one indirect DMA pulls TPS*128 x-rows (128B each) from HBM
     segment : per tile, matmul zT[32, off:off+M] += gath[128,32].T @ bval[128,M]
               (TensorE does gather-weighted segment-sum; PSUM accumulates)
     transform: out = zT.T @ W per 128-row chunk (no transposes needed)
     store   : one DMA writes the 512 finished rows.
 - Host un-permutes the packed rows into the final [100000, 32] output.
"""

import sys
import types

import numpy as np


def _install_ntff_hook_shim():
    """bass_utils' axon trace path imports antenv.axon_hooks, which this
    container image lacks.  Provide it (with the real ctypes-based profiler
    hook when available) so BASS_TRACE=1 in the environment doesn't crash."""
    if "antenv.axon_hooks" in sys.modules:
        return
    mod = types.ModuleType("antenv.axon_hooks")
    _h = [None]
    mod.set_axon_ntff_profile_hook = lambda h: _h.__setitem__(0, h)
    mod.get_axon_ntff_profile_hook = lambda: _h[0]
    sys.modules["antenv.axon_hooks"] = mod
    try:
        from trn_agent_boot.trn_boot import _ntff_profile_via_ctypes

        mod.set_axon_ntff_profile_hook(
            _ntff_profile_via_ctypes("/opt/axon/libaxon_pjrt.so")
        )
    except Exception:
        pass


_install_ntff_hook_shim()

import concourse.bass as bass
import concourse.mybir as mybir
import concourse.tile as tile
from concourse import bacc
from concourse.bass import IndirectOffsetOnAxis
from concourse.bass_utils import run_bass_kernel_spmd

N_NODES = 100_000
N_CORES = 8
RPC = N_NODES // N_CORES  # rows per core
F = 32
M_FIX = 16                # output rows (bval columns) per tile
TPS = 32                  # tiles per super-block
RPS = M_FIX * TPS         # 512 output rows per super-block
P = 128

f32 = mybir.dt.float32
i32 = mybir.dt.int32

_compiled_cache = {}


DMAX = 120  # max edges per fragment (rows above this split into fragments)


def _pack_frags(fdeg):
    """Bin-pack fragments into tiles: <=128 slots, <=M_FIX fragments per tile.

    Greedy: repeatedly take the largest-degree fragment that still fits.
    Returns a list of fragment-id lists.
    """
    maxd = int(fdeg.max()) if fdeg.size else 0
    by_deg = [np.where(fdeg == deg)[0] for deg in range(maxd + 1)]
    ptrs = [0] * (maxd + 1)
    navail = int((fdeg > 0).sum())
    bins = []
    cur = maxd
    while navail:
        slots = 0
        binrows = []
        while len(binrows) < M_FIX and navail:
            while cur > 0 and ptrs[cur] >= len(by_deg[cur]):
                cur -= 1
            dd = min(128 - slots, cur)
            while dd > 0 and ptrs[dd] >= len(by_deg[dd]):
                dd -= 1
            if dd <= 0:
                break
            r = by_deg[dd][ptrs[dd]]
            ptrs[dd] += 1
            navail -= 1
            slots += dd
            binrows.append(int(r))
        bins.append(binrows)
    return bins


def _prepare_core(rows, cols, vals, core):
    """Fragment this core's rows and bin-pack the fragments."""
    lo = core * RPC
    hi = lo + RPC
    # edges are sorted by destination row
    bounds = np.searchsorted(rows, np.arange(lo, hi + 1))
    starts = bounds[:-1]
    d = (bounds[1:] - bounds[:-1]).astype(np.int64)
    # fragments: (local_row, abs_edge_start, deg); rows with d > DMAX split
    frow, fstart, fdeg = [], [], []
    for r in range(RPC):
        deg = int(d[r])
        if deg == 0:
            continue
        s0 = int(starts[r])
        while deg > 0:
            take = min(deg, DMAX)
            frow.append(r)
            fstart.append(s0)
            fdeg.append(take)
            s0 += take
            deg -= take
    frow = np.asarray(frow, np.int64)
    fstart = np.asarray(fstart, np.int64)
    fdeg = np.asarray(fdeg, np.int64)
    bins = _pack_frags(fdeg)
    return bins, frow, fstart, fdeg


def _assemble_core(bins, frow, fstart, fdeg, cols, vals, nt):
    idx_all = np.zeros((P, nt), np.int32)
    bval_all = np.zeros((P, nt * M_FIX), np.float32)
    fpos = np.full(len(frow), -1, np.int64)
    cols32 = cols.astype(np.int32, copy=False)
    vals32 = vals.astype(np.float32, copy=False)
    for t, binfrags in enumerate(bins):
        base = 0
        for i, fr in enumerate(binfrags):
            deg = int(fdeg[fr])
            s0 = int(fstart[fr])
            idx_all[base : base + deg, t] = cols32[s0 : s0 + deg]
            bval_all[base : base + deg, t * M_FIX + i] = vals32[s0 : s0 + deg]
            fpos[fr] = t * M_FIX + i
            base += deg
    return idx_all, bval_all, fpos


def _build_program(nsb):
    nt = nsb * TPS
    nrows = nt * M_FIX
    nc = bacc.Bacc("TRN2", target_bir_lowering=False, debug=False)
    x = nc.dram_tensor("x", [N_NODES, F], f32, kind="ExternalInput")
    idx = nc.dram_tensor("idx", [P, nt], i32, kind="ExternalInput")
    bval = nc.dram_tensor("bval", [P, nrows], f32, kind="ExternalInput")
    w = nc.dram_tensor("w", [F, F], f32, kind="ExternalInput")
    out = nc.dram_tensor("out", [nrows, F], f32, kind="ExternalOutput")

    with tile.TileContext(nc) as tc:
        with (
            tc.tile_pool(name="const", bufs=1) as cpool,
            tc.tile_pool(name="meta", bufs=4) as mpool,
            tc.tile_pool(name="gath", bufs=3) as gpool,
            tc.tile_pool(name="zt", bufs=3, space="PSUM") as ztpool,
            tc.tile_pool(name="po", bufs=2, space="PSUM") as popool,
            tc.tile_pool(name="outp", bufs=3) as opool,
        ):
            wt = cpool.tile([F, F], f32)
            nc.sync.dma_start(wt[:], w[:])
            for sb in range(nsb):
                idx_t = mpool.tile([P, TPS], i32, tag="idx")
                nc.sync.dma_start(idx_t[:], idx[:, sb * TPS : (sb + 1) * TPS])
                bval_t = mpool.tile([P, RPS], f32, tag="bval")
                nc.sync.dma_start(bval_t[:], bval[:, sb * RPS : (sb + 1) * RPS])
                zt = ztpool.tile([F, RPS], f32, tag="zt")
                gath = gpool.tile([P, TPS * F], f32, tag="gath")
                for t in range(TPS):
                    # HW-supported indirect mode: 128 per-partition offsets,
                    # one x-row (128B) per partition.  All TPS gathers write
                    # slices of one tile so Tile emits slot waits once per
                    # super-block, not per gather.
                    nc.gpsimd.indirect_dma_start(
                        out=gath[:, t * F : (t + 1) * F],
                        out_offset=None,
                        in_=x[:],
                        in_offset=IndirectOffsetOnAxis(
                            ap=idx_t[:, t : t + 1], axis=0
                        ),
                    )
                for t in range(TPS):
                    nc.tensor.matmul(
                        out=zt[:, t * M_FIX : (t + 1) * M_FIX],
                        lhsT=gath[:, t * F : (t + 1) * F],
                        rhs=bval_t[:, t * M_FIX : (t + 1) * M_FIX],
                        start=True,
                        stop=True,
                    )
                zt_sb = opool.tile([F, RPS], f32, tag="ztsb")
                nc.vector.tensor_copy(zt_sb[:], zt[:])
                po = popool.tile([P, (RPS // P) * F], f32, tag="po")
                for c in range(RPS // P):
                    nc.tensor.matmul(
                        out=po[:, c * F : (c + 1) * F],
                        lhsT=zt_sb[:, c * P : (c + 1) * P],
                        rhs=wt[:],
                        start=True,
                        stop=True,
                    )
                ot = opool.tile([P, (RPS // P) * F], f32, tag="ot")
                nc.vector.tensor_copy(ot[:], po[:])
                nc.sync.dma_start(
                    out[sb * RPS : (sb + 1) * RPS, :].rearrange(
                        "(c p) f -> p c f", p=P
                    ),
                    ot[:].rearrange("p (c f) -> p c f", f=F),
                )
    nc.compile()
    return nc


def kernel(x, rows, cols, vals, weight):
    x = np.ascontiguousarray(np.asarray(x, dtype=np.float32))
    rows = np.asarray(rows)
    cols = np.asarray(cols)
    vals = np.asarray(vals, dtype=np.float32)
    weight = np.ascontiguousarray(np.asarray(weight, dtype=np.float32))

    per_core = [_prepare_core(rows, cols, vals, k) for k in range(N_CORES)]
    max_bins = max(len(pc[0]) for pc in per_core)
    nsb = (max_bins + TPS - 1) // TPS
    nt = nsb * TPS

    if nsb not in _compiled_cache:
        _compiled_cache[nsb] = _build_program(nsb)
    nc = _compiled_cache[nsb]

    in_maps = []
    poss = []
    for k in range(N_CORES):
        bins, frow, fstart, fdeg = per_core[k]
        idx_all, bval_all, fpos = _assemble_core(
            bins, frow, fstart, fdeg, cols, vals, nt
        )
        poss.append((frow, fpos))
        in_maps.append({"x": x, "idx": idx_all, "bval": bval_all, "w": weight})

    res = run_bass_kernel_spmd(nc, in_maps, list(range(N_CORES)))

    out_full = np.zeros((N_NODES, F), np.float32)
    for k in range(N_CORES):
        dev = res.results[k]["out"]
        frow, fpos = poss[k]
        # rows split into multiple fragments accumulate; others assign once
        np.add.at(out_full, k * RPC + frow, dev[fpos])
    return out_full


# revision 12
# speedup vs baseline: 1.0077x; 1.0077x over previous
"""GCNConv (COO SpMM + feature transform) distributed over 8 NeuronCores.

out = segment_sum(x[cols] * vals, rows) @ weight

Strategy (1D row partition of the sparse matrix, per the CAGNET-style hint):
 - Destination rows are split into 8 contiguous blocks of 12500 rows; core k
   owns rows [12500k, 12500(k+1)) and the edges targeting them (edges arrive
   sorted by destination row).
 - x (the gather table) and the 32x32 weight are replicated per core.
 - Host-side (inside kernel(), numpy): each core's rows are bin-packed into
   "tiles" of <=128 edge slots / <=M_FIX rows.  For each tile we build
     idx[p]  : source node of edge-slot p   (gather index)
     bval[p, i] = val(edge) if slot p belongs to tile-row i else 0
   i.e. bval is the one-hot segment-sum matrix with the edge weights folded
   in, fully precomputed on host.
 - Device: per super-block of TPS tiles
     gather  : one indirect DMA pulls TPS*128 x-rows (128B each) from HBM
     segment : per tile, matmul zT[32, off:off+M] += gath[128,32].T @ bval[128,M]
               (TensorE does gather-weighted segment-sum; PSUM accumulates)
     transform: out = zT.T @ W per 128-row chunk (no transposes needed)
     store   : one DMA writes the 512 finished rows.
 - Host un-permutes the packed rows into the final [100000, 32] output.
"""

import sys
import types

import numpy as np


def _install_ntff_hook_shim():
    """bass_utils' axon trace path imports antenv.axon_hooks, which this
    container image lacks.  Provide it (with the real ctypes-based profiler
    hook when available) so BASS_TRACE=1 in the environment doesn't crash."""
    if "antenv.axon_hooks" in sys.modules:
        return
    mod = types.ModuleType("antenv.axon_hooks")
    _h = [None]
    mod.set_axon_ntff_profile_hook = lambda h: _h.__setitem__(0, h)
    mod.get_axon_ntff_profile_hook = lambda: _h[0]
    sys.modules["antenv.axon_hooks"] = mod
    try:
        from trn_agent_boot.trn_boot import _ntff_profile_via_ctypes

        mod.set_axon_ntff_profile_hook(
            _ntff_profile_via_ctypes("/opt/axon/libaxon_pjrt.so")
        )
    except Exception:
        pass


_install_ntff_hook_shim()

import concourse.bass as bass
import concourse.mybir as mybir
import concourse.tile as tile
from concourse import bacc
from concourse.bass import IndirectOffsetOnAxis
from concourse.bass_utils import run_bass_kernel_spmd

N_NODES = 100_000
N_CORES = 8
RPC = N_NODES // N_CORES  # rows per core
F = 32
M_FIX = 16                # output rows (bval columns) per tile
TPS = 32                  # tiles per super-block
RPS = M_FIX * TPS         # 512 output rows per super-block
P = 128

f32 = mybir.dt.float32
i32 = mybir.dt.int32

_compiled_cache = {}


DMAX = 120  # max edges per fragment (rows above this split into fragments)


def _pack_frags(fdeg):
    """Bin-pack fragments into tiles: <=128 slots, <=M_FIX fragments per tile.

    Greedy: repeatedly take the largest-degree fragment that still fits.
    Returns a list of fragment-id lists.
    """
    maxd = int(fdeg.max()) if fdeg.size else 0
    by_deg = [np.where(fdeg == deg)[0] for deg in range(maxd + 1)]
    ptrs = [0] * (maxd + 1)
    navail = int((fdeg > 0).sum())
    bins = []
    cur = maxd
    while navail:
        slots = 0
        binrows = []
        while len(binrows) < M_FIX and navail:
            while cur > 0 and ptrs[cur] >= len(by_deg[cur]):
                cur -= 1
            dd = min(128 - slots, cur)
            while dd > 0 and ptrs[dd] >= len(by_deg[dd]):
                dd -= 1
            if dd <= 0:
                break
            r = by_deg[dd][ptrs[dd]]
            ptrs[dd] += 1
            navail -= 1
            slots += dd
            binrows.append(int(r))
        bins.append(binrows)
    return bins


def _prepare_core(rows, cols, vals, core):
    """Fragment this core's rows and bin-pack the fragments."""
    lo = core * RPC
    hi = lo + RPC
    # edges are sorted by destination row
    bounds = np.searchsorted(rows, np.arange(lo, hi + 1))
    starts = bounds[:-1]
    d = (bounds[1:] - bounds[:-1]).astype(np.int64)
    # fragments: (local_row, abs_edge_start, deg); rows with d > DMAX split
    frow, fstart, fdeg = [], [], []
    for r in range(RPC):
        deg = int(d[r])
        if deg == 0:
            continue
        s0 = int(starts[r])
        while deg > 0:
            take = min(deg, DMAX)
            frow.append(r)
            fstart.append(s0)
            fdeg.append(take)
            s0 += take
            deg -= take
    frow = np.asarray(frow, np.int64)
    fstart = np.asarray(fstart, np.int64)
    fdeg = np.asarray(fdeg, np.int64)
    bins = _pack_frags(fdeg)
    return bins, frow, fstart, fdeg


def _assemble_core(bins, frow, fstart, fdeg, cols, vals, nt):
    idx_all = np.zeros((P, nt), np.int32)
    bval_all = np.zeros((P, nt * M_FIX), np.float32)
    fpos = np.full(len(frow), -1, np.int64)
    cols32 = cols.astype(np.int32, copy=False)
    vals32 = vals.astype(np.float32, copy=False)
    for t, binfrags in enumerate(bins):
        base = 0
        for i, fr in enumerate(binfrags):
            deg = int(fdeg[fr])
            s0 = int(fstart[fr])
            idx_all[base : base + deg, t] = cols32[s0 : s0 + deg]
            bval_all[base : base + deg, t * M_FIX + i] = vals32[s0 : s0 + deg]
            fpos[fr] = t * M_FIX + i
            base += deg
    return idx_all, bval_all, fpos


def _build_program(nsb):
    nt = nsb * TPS
    nrows = nt * M_FIX
    nc = bacc.Bacc("TRN2", target_bir_lowering=False, debug=False)
    x = nc.dram_tensor("x", [N_NODES, F], f32, kind="ExternalInput")
    idx = nc.dram_tensor("idx", [P, nt], i32, kind="ExternalInput")
    bval = nc.dram_tensor("bval", [P, nrows], f32, kind="ExternalInput")
    w = nc.dram_tensor("w", [F, F], f32, kind="ExternalInput")
    out = nc.dram_tensor("out", [nrows, F], f32, kind="ExternalOutput")

    with tile.TileContext(nc) as tc:
        with (
            tc.tile_pool(name="const", bufs=1) as cpool,
            tc.tile_pool(name="meta", bufs=4) as mpool,
            tc.tile_pool(name="gath", bufs=16) as gpool,
            tc.tile_pool(name="zt", bufs=3, space="PSUM") as ztpool,
            tc.tile_pool(name="po", bufs=2, space="PSUM") as popool,
            tc.tile_pool(name="outp", bufs=3) as opool,
        ):
            wt = cpool.tile([F, F], f32)
            nc.sync.dma_start(wt[:], w[:])
            for sb in range(nsb):
                idx_t = mpool.tile([P, TPS], i32, tag="idx")
                nc.sync.dma_start(idx_t[:], idx[:, sb * TPS : (sb + 1) * TPS])
                bval_t = mpool.tile([P, RPS], f32, tag="bval")
                nc.sync.dma_start(bval_t[:], bval[:, sb * RPS : (sb + 1) * RPS])
                zt = ztpool.tile([F, RPS], f32, tag="zt")
                for t in range(TPS):
                    # HW-supported indirect mode: 128 per-partition offsets,
                    # one x-row (128B) per partition.
                    gath = gpool.tile([P, F], f32, tag="gath")
                    nc.gpsimd.indirect_dma_start(
                        out=gath[:],
                        out_offset=None,
                        in_=x[:],
                        in_offset=IndirectOffsetOnAxis(
                            ap=idx_t[:, t : t + 1], axis=0
                        ),
                    )
                    nc.tensor.matmul(
                        out=zt[:, t * M_FIX : (t + 1) * M_FIX],
                        lhsT=gath[:],
                        rhs=bval_t[:, t * M_FIX : (t + 1) * M_FIX],
                        start=True,
                        stop=True,
                    )
                zt_sb = opool.tile([F, RPS], f32, tag="ztsb")
                nc.vector.tensor_copy(zt_sb[:], zt[:])
                po = popool.tile([P, (RPS // P) * F], f32, tag="po")
                for c in range(RPS // P):
                    nc.tensor.matmul(
                        out=po[:, c * F : (c + 1) * F],
                        lhsT=zt_sb[:, c * P : (c + 1) * P],
                        rhs=wt[:],
                        start=True,
                        stop=True,
                    )
                ot = opool.tile([P, (RPS // P) * F], f32, tag="ot")
                nc.vector.tensor_copy(ot[:], po[:])
                nc.sync.dma_start(
                    out[sb * RPS : (sb + 1) * RPS, :].rearrange(
                        "(c p) f -> p c f", p=P
                    ),
                    ot[:].rearrange("p (c f) -> p c f", f=F),
                )
    nc.compile()
    return nc


def kernel(x, rows, cols, vals, weight):
    x = np.ascontiguousarray(np.asarray(x, dtype=np.float32))
    rows = np.asarray(rows)
    cols = np.asarray(cols)
    vals = np.asarray(vals, dtype=np.float32)
    weight = np.ascontiguousarray(np.asarray(weight, dtype=np.float32))

    per_core = [_prepare_core(rows, cols, vals, k) for k in range(N_CORES)]
    max_bins = max(len(pc[0]) for pc in per_core)
    nsb = (max_bins + TPS - 1) // TPS
    nt = nsb * TPS

    if nsb not in _compiled_cache:
        _compiled_cache[nsb] = _build_program(nsb)
    nc = _compiled_cache[nsb]

    in_maps = []
    poss = []
    for k in range(N_CORES):
        bins, frow, fstart, fdeg = per_core[k]
        idx_all, bval_all, fpos = _assemble_core(
            bins, frow, fstart, fdeg, cols, vals, nt
        )
        poss.append((frow, fpos))
        in_maps.append({"x": x, "idx": idx_all, "bval": bval_all, "w": weight})

    res = run_bass_kernel_spmd(nc, in_maps, list(range(N_CORES)))

    out_full = np.zeros((N_NODES, F), np.float32)
    for k in range(N_CORES):
        dev = res.results[k]["out"]
        frow, fpos = poss[k]
        # rows split into multiple fragments accumulate; others assign once
        np.add.at(out_full, k * RPC + frow, dev[fpos])
    return out_full
